# revision 27
# baseline (speedup 1.0000x reference)
"""Trainium2 Bass kernel for a SuperGlue-style AttentionalGNN
(12 layers alternating self/cross attention, D=256, 4 heads, B=2, N=M=2048).

Sharding (8 NeuronCores): batch b = core//4 per 4-core group; within the
group each core owns a 512-position slice (h4 = core%4) of BOTH descriptor
streams. Every core keeps full fp32 replicas of both streams of its batch
(K/V/attention sources) plus fp32 masters of its own positions (residual
chain stays fp32). The replicas are built from the per-core slices by an
AllGather before layer 0, and refreshed by an AllGather after each layer.

Attention per stream per layer:
  pass 1 (layers 8..11): scores[q, m] via row-packed K=64 matmuls, DVE
    free-axis max-reduce -> per-query shift u = -8*max, scattered
    into row 64 of q_aug via small DMAs.
  pass 2: scoresT[m, q] recomputed with the shift folded in as a 65th
    contraction row (k_aug row 64 = ones, q_aug row 64 = u), ACT
    exp(scale=1/8) -> unnormalized probs; PV matmul with a ones
    column in vT_aug producing the softmax denominator as msgU row 64;
    normalize via reciprocal + PE ones-broadcast + fused multiply.
  Layers 0..7 skip pass 1 (scores bounded, raw exp safe; pass 2 then uses
  row-packed K=64 matmuls).

Host-side folding (float64): head-major channel permutation; k-bias dropped
(softmax shift invariance); v-bias folded into the merge bias; merge
projection folded into the MLP first layer (W1bm = W1[:,256:] @ Wm);
batch-norm affine folded into a relu(scale*x + beta') epilogue.

Dispatch: a persistent jit (built once per process) runs the NEFF on all 8
cores via shard_map. Weights are uploaded to the devices once and cached
there (validated by a content hash); each call ships only the per-core
input slices (fp16) and fetches the per-core output slices as int8 codes
plus the per-row fp32 scale r = 127/max|v| that the device actually used
(host dequantizes by dividing by r, so the reciprocal-approx error
cancels). The on-device pipeline stays fp32. The donated output buffers
are generated on-device once and then recycled from the previous call's
output (the NEFF overwrites every element, so their contents are
irrelevant).
"""

import os
import zlib

import numpy as np

L, NH, D, DH = 12, 4, 256, 64
B, N = 2, 2048
NLOC = 512
EPS = 1e-5
SHIFT_LAYERS = set(range(8, 12))
L_RUN = int(os.environ.get("L_RUN", "12"))
NCORE = 8

_COMPILED = {}


def _prep_host(Wq, bq, Wk, bk, Wv, bv, Wm, bm, W1, b1, gamma, beta, W2, b2):
    f8 = np.float64
    idx = np.array([(hm % DH) * NH + hm // DH for hm in range(D)])

    Wqp = Wq[:, idx, :].astype(f8)
    bqp = bq[:, idx].astype(f8)
    Wkp = Wk[:, idx, :].astype(f8)
    Wvp = Wv[:, idx, :].astype(f8)
    bvp = bv[:, idx].astype(f8)
    Wmp = Wm[:, :, idx].astype(f8)

    W1 = W1.astype(f8)
    W1a = W1[:, :, :D]
    W1b = W1[:, :, D:]
    W1bm = np.einsum('lij,ljk->lik', W1b, Wmp)
    bm_f = np.einsum('lij,lj->li', Wmp, bvp) + bm.astype(f8)
    b1f = b1.astype(f8) + np.einsum('lij,lj->li', W1b, bm_f)
    scale = gamma.astype(f8) * np.float64(1.0 / np.sqrt(1.0 + EPS))
    beta_f = scale * b1f + beta.astype(f8)
    W2 = W2.astype(f8)

    Wpack = np.zeros((L, 128, 4608), dtype=np.float32)
    for i in range(L):
        cols = []
        for blkmat, nblk in ((Wqp[i].T, 2), (Wkp[i].T, 2), (Wvp[i].T, 2),
                             (W1a[i].T, 2), (W1bm[i].T, 2), (W2[i].T, 4)):
            for kblk in range(nblk):
                cols.append(blkmat[kblk * 128:(kblk + 1) * 128, :])
        Wpack[i] = np.concatenate(cols, axis=1).astype(np.float32)

    BIAS = np.zeros((128, L * 12), dtype=np.float32)
    for i in range(L):
        o = i * 12
        BIAS[:, o + 0] = bqp[i][:128]
        BIAS[:, o + 1] = bqp[i][128:]
        for c in range(4):
            BIAS[:, o + 2 + c] = scale[i][c * 128:(c + 1) * 128]
            BIAS[:, o + 6 + c] = beta_f[i][c * 128:(c + 1) * 128]
        BIAS[:, o + 10] = b2[i][:128]
        BIAS[:, o + 11] = b2[i][128:]
    return Wpack, BIAS


def _build():
    import concourse.bass as bass  # noqa: F401
    import concourse.mybir as mybir
    from concourse import tile, bacc

    F32 = mybir.dt.float32
    F16 = mybir.dt.float16
    I8 = mybir.dt.int8
    AX = mybir.AxisListType
    ALU = mybir.AluOpType
    ACTF = mybir.ActivationFunctionType

    nc = bacc.Bacc("TRN2", target_bir_lowering=False, debug=False, num_devices=8)

    m_d = nc.declare_dram_parameter("m", [2, D, NLOC], F16, isOutput=False)
    W_d = nc.declare_dram_parameter("W", [L, 128, 4608], F32, isOutput=False)
    B_d = nc.declare_dram_parameter("BIAS", [128, L * 12], F32, isOutput=False)
    out_d = nc.declare_dram_parameter("out", [2, D, NLOC], I8, isOutput=True)
    osc_d = nc.declare_dram_parameter("osc", [128, 4], F32, isOutput=True)

    RG = [[0, 1, 2, 3], [4, 5, 6, 7]]
    QT = [0, 256]
    KT = [512, 768]
    VT = [1024, 1280]
    W1AT = [1536, 2048]
    W1BT = [2560, 3072]
    W2T = [3584, 3840, 4096, 4352]

    with tile.TileContext(nc) as tc:
        with (
            tc.tile_pool(name="state", bufs=1) as st,
            tc.tile_pool(name="wpool", bufs=2) as wp,
            tc.tile_pool(name="work", bufs=2) as wk,
            tc.tile_pool(name="probp", bufs=3) as pp,
            tc.tile_pool(name="psA", bufs=2, space="PSUM") as psA,
            tc.tile_pool(name="psB", bufs=4, space="PSUM") as psB,
            tc.tile_pool(name="dram", bufs=2, space="DRAM") as dr,
        ):
            big = lambda: psA.tile([128, 1024], F32, tag="big", name="big")
            small = lambda: psB.tile([128, NLOC], F32, tag="small", name="small")

            bias_all = st.tile([128, L * 12], F32, tag="bias")
            nc.sync.dma_start(bias_all[:], B_d[:])
            ones64 = st.tile([1, 64], F32, tag="ones64")
            nc.vector.memset(ones64[:], 1.0)

            xr = [[st.tile([128, N], F32, tag=f"xr{s}{kk}", name=f"xr{s}{kk}") for kk in range(2)]
                  for s in range(2)]
            xm = [[st.tile([128, NLOC], F32, tag=f"xm{s}{kk}", name=f"xm{s}{kk}") for kk in range(2)]
                  for s in range(2)]

            for s in range(2):
                for kk in range(2):
                    mh = st.tile([128, NLOC], F16, tag=f"mh{s}{kk}", name=f"mh{s}{kk}")
                    nc.sync.dma_start(mh[:], m_d[s, kk * 128:(kk + 1) * 128, :])
                    nc.vector.tensor_copy(xm[s][kk][:], mh[:])

            def replica_update(agin, agout):
                nc.gpsimd.collective_compute(
                    "AllGather", mybir.AluOpType.bypass, replica_groups=RG,
                    ins=[agin.opt()], outs=[agout.opt()])
                ag5 = agout[:].rearrange("(r s k p) c -> r s k p c", r=4, s=2, k=2, p=128)
                for s in range(2):
                    for kk in range(2):
                        srcv = ag5[:, s, kk, :, :].transpose([1, 0, 2])
                        dstv = xr[s][kk][:].rearrange("p (r c) -> p r c", r=4, c=NLOC)
                        nc.sync.dma_start(dstv, srcv)

            # build the full-stream replicas from the per-core slices
            agin0 = dr.tile([2 * D, NLOC], F32, tag="agin")
            agout0 = dr.tile([4 * 2 * D, NLOC], F32, tag="agout")
            for s in range(2):
                for kk in range(2):
                    nc.sync.dma_start(
                        agin0[s * D + kk * 128: s * D + (kk + 1) * 128, :], xm[s][kk][:])
            replica_update(agin0, agout0)

            for li in range(L_RUN):
                shift = li in SHIFT_LAYERS
                wt = wp.tile([128, 4608], F32, tag="w")
                nc.sync.dma_start(wt[:], W_d[li])
                bcol = lambda c: bias_all[:, li * 12 + c:li * 12 + c + 1]

                agin = dr.tile([2 * D, NLOC], F32, tag="agin")
                agout = dr.tile([4 * 2 * D, NLOC], F32, tag="agout")

                for s in range(2):
                    src = xr[s] if li % 2 == 0 else xr[1 - s]

                    # ---------- projections ----------
                    qp, qa = [], []
                    for o in range(2):
                        psq = small()
                        for kk in range(2):
                            nc.tensor.matmul(
                                psq[:], wt[:, QT[kk] + o * 128: QT[kk] + (o + 1) * 128],
                                xm[s][kk][:], start=(kk == 0), stop=(kk == 1))
                        qpo = wk.tile([128, NLOC], F32, tag="qp")
                        nc.vector.tensor_scalar(qpo[:], psq[:], bcol(o), None, op0=ALU.add)
                        qp.append(qpo)
                    if shift:
                        for h in range(NH):
                            t_ = wk.tile([65, NLOC], F32, tag=f"qa{h}", name=f"qa{h}")
                            nc.vector.tensor_copy(
                                t_[0:64, :], qp[h // 2][(h % 2) * 64:(h % 2) * 64 + 64, :])
                            qa.append(t_)

                    kp, ka = [], []
                    for o in range(2):
                        kpo = wk.tile([128, N], F32, tag=f"kp{o}", name=f"kp{o}", bufs=1)
                        for mc4 in range(4):
                            psk = small()
                            for kk in range(2):
                                nc.tensor.matmul(
                                    psk[:], wt[:, KT[kk] + o * 128: KT[kk] + (o + 1) * 128],
                                    src[kk][:, mc4 * 512:(mc4 + 1) * 512],
                                    start=(kk == 0), stop=(kk == 1))
                            nc.vector.tensor_copy(kpo[:, mc4 * 512:(mc4 + 1) * 512], psk[:])
                        kp.append(kpo)
                    if shift:
                        for h in range(NH):
                            t_ = wk.tile([65, N], F32, tag=f"ka{h}", name=f"ka{h}", bufs=1)
                            nc.vector.tensor_copy(
                                t_[0:64, :], kp[h // 2][(h % 2) * 64:(h % 2) * 64 + 64, :])
                            nc.vector.memset(t_[64:65, :], 1.0)
                            ka.append(t_)

                    va = []
                    for mc in range(16):
                        psv = small()
                        for kk in range(2):
                            nc.tensor.matmul(
                                psv[:, 0:256], src[kk][:, mc * 128:(mc + 1) * 128],
                                wt[:, VT[kk]:VT[kk] + 256],
                                start=(kk == 0), stop=(kk == 1))
                        t_ = wk.tile([128, 260], F32, tag=f"va{mc}", name=f"va{mc}", bufs=1)
                        dst = t_[:].rearrange("p (h d) -> p h d", h=4, d=65)[:, :, 0:64]
                        srcv = psv[:, 0:256].rearrange("p (h d) -> p h d", h=4, d=64)
                        nc.vector.tensor_copy(dst, srcv)
                        nc.vector.memset(t_[:, 64:260:65], 1.0)
                        va.append(t_)

                    # ---------- pass 1: per-query max (f32, unpacked) ----------
                    if shift:
                        u8 = wk.tile([128, 16], F32, tag="u8")
                        for h in range(NH):
                            o, hpar = h // 2, h % 2
                            sl = slice(hpar * 64, hpar * 64 + 64)
                            for t4 in range(4):
                                uparts = wk.tile([128, 4], F32, tag="uparts", name="uparts")
                                for mq in range(4):
                                    ps1 = psB.tile([128, NLOC], F32, tag="small", name="ps1")
                                    nc.tensor.matmul(
                                        ps1[:], qp[o][sl, t4 * 128:(t4 + 1) * 128],
                                        kp[o][sl, mq * 512:(mq + 1) * 512],
                                        start=True, stop=True,
                                        tile_position=(hpar * 64, 0))
                                    nc.vector.tensor_reduce(
                                        uparts[:, mq:mq + 1], ps1[:],
                                        axis=AX.X, op=ALU.max, negate=True)
                                nc.vector.tensor_reduce(
                                    u8[:, h * 4 + t4: h * 4 + t4 + 1],
                                    uparts[:], axis=AX.X, op=ALU.min)
                        for h in range(NH):
                            for t4 in range(4):
                                nc.sync.dma_start(
                                    qa[h][64:65, t4 * 128:(t4 + 1) * 128],
                                    u8[:, h * 4 + t4: h * 4 + t4 + 1])

                    # ---------- pass 2 + PV ----------
                    msgt = []
                    for o in range(2):
                        h0, h1 = 2 * o, 2 * o + 1
                        mU0 = psB.tile([65, NLOC], F32, tag="small", name="mU0")
                        mU1 = psB.tile([65, NLOC], F32, tag="small", name="mU1")
                        for mc in range(16):
                            qk2 = big()
                            if shift:
                                nc.tensor.matmul(
                                    qk2[:, 0:512], ka[h0][:, mc * 128:(mc + 1) * 128],
                                    qa[h0][:], start=True, stop=True)
                                nc.tensor.matmul(
                                    qk2[:, 512:1024], ka[h1][:, mc * 128:(mc + 1) * 128],
                                    qa[h1][:], start=True, stop=True)
                            else:
                                nc.tensor.matmul(
                                    qk2[:, 0:512],
                                    kp[o][0:64, mc * 128:(mc + 1) * 128],
                                    qp[o][0:64, :], start=True, stop=True,
                                    tile_position=(0, 0))
                                nc.tensor.matmul(
                                    qk2[:, 512:1024],
                                    kp[o][64:128, mc * 128:(mc + 1) * 128],
                                    qp[o][64:128, :], start=True, stop=True,
                                    tile_position=(64, 0))
                            probt = pp.tile([128, 1024], F32, tag="probt", bufs=2)
                            nc.scalar.activation(probt[:], qk2[:], ACTF.Exp, scale=0.125)
                            nc.tensor.matmul(mU0[:65, :], va[mc][:, 65 * h0:65 * h0 + 65],
                                             probt[:, 0:512], start=(mc == 0), stop=(mc == 15))
                            nc.tensor.matmul(mU1[:65, :], va[mc][:, 65 * h1:65 * h1 + 65],
                                             probt[:, 512:1024], start=(mc == 0), stop=(mc == 15))
                        mo = wk.tile([128, NLOC], F32, tag="msg")
                        for hh, mU in ((0, mU0), (1, mU1)):
                            zr = wk.tile([1, NLOC], F32, tag="zr")
                            nc.vector.tensor_copy(zr[:], mU[64:65, :])
                            rz = wk.tile([1, NLOC], F32, tag="rz")
                            nc.vector.reciprocal_approx_fast(rz[:], zr[:])
                            rzp = psB.tile([64, NLOC], F32, tag="small", name="rzp")
                            nc.tensor.matmul(rzp[:], ones64[:], rz[:], start=True, stop=True)
                            rzs = wk.tile([64, NLOC], F32, tag="rzs")
                            nc.vector.tensor_copy(rzs[:], rzp[:])
                            nc.vector.tensor_tensor(
                                mo[hh * 64:hh * 64 + 64, :], mU[0:64, :], rzs[:],
                                op=ALU.mult)
                        msgt.append(mo)

                    # ---------- MLP ----------
                    hb = []
                    for m4 in range(4):
                        hps = small()
                        nc.tensor.matmul(
                            hps[:], wt[:, W1AT[0] + m4 * 128: W1AT[0] + (m4 + 1) * 128],
                            xm[s][0][:], start=True, stop=False)
                        nc.tensor.matmul(
                            hps[:], wt[:, W1BT[0] + m4 * 128: W1BT[0] + (m4 + 1) * 128],
                            msgt[0][:], start=False, stop=False)
                        nc.tensor.matmul(
                            hps[:], wt[:, W1AT[1] + m4 * 128: W1AT[1] + (m4 + 1) * 128],
                            xm[s][1][:], start=False, stop=False)
                        nc.tensor.matmul(
                            hps[:], wt[:, W1BT[1] + m4 * 128: W1BT[1] + (m4 + 1) * 128],
                            msgt[1][:], start=False, stop=True)
                        hbt = wk.tile([128, NLOC], F32, tag=f"hb{m4}", bufs=1)
                        nc.scalar.activation(hbt[:], hps[:], ACTF.Relu,
                                             bias=bcol(6 + m4), scale=bcol(2 + m4))
                        hb.append(hbt)

                    for o2 in range(2):
                        dps = small()
                        for kk4 in range(4):
                            nc.tensor.matmul(
                                dps[:], wt[:, W2T[kk4] + o2 * 128: W2T[kk4] + (o2 + 1) * 128],
                                hb[kk4][:], start=(kk4 == 0), stop=(kk4 == 3))
                        nc.vector.affine_then_add(
                            xm[s][o2][:], dps[:], xm[s][o2][:], 1.0, bcol(10 + o2))
                        if li < L_RUN - 1:
                            nc.sync.dma_start(
                                agin[s * D + o2 * 128: s * D + (o2 + 1) * 128, :],
                                xm[s][o2][:])

                # ---------- collective + replica update ----------
                # (the final layer's gather is dead: outputs come from xm)
                if li < L_RUN - 1:
                    replica_update(agin, agout)

            # int8 output: per-partition-row scale r = 127/max|v|; ship both
            # the int8 codes and the exact r used (host divides by r).
            osc = st.tile([128, 4], F32, tag="osc")
            for s in range(2):
                for o2 in range(2):
                    am = wk.tile([128, 1], F32, tag="am")
                    nc.vector.tensor_reduce(
                        am[:], xm[s][o2][:], axis=AX.X, op=ALU.max,
                        apply_absolute_value=True)
                    am2 = wk.tile([128, 1], F32, tag="am2")
                    nc.vector.tensor_scalar_max(am2[:], am[:], 1e-12)
                    rra = wk.tile([128, 1], F32, tag="rra")
                    nc.vector.reciprocal_approx_fast(rra[:], am2[:])
                    rr = wk.tile([128, 1], F32, tag="rr")
                    nc.vector.tensor_scalar_mul(rr[:], rra[:], 127.0)
                    nc.vector.tensor_copy(osc[:, s * 2 + o2: s * 2 + o2 + 1], rr[:])
                    tq = wk.tile([128, NLOC], F32, tag="tq")
                    nc.vector.tensor_scalar(tq[:], xm[s][o2][:], rr[:, 0:1], 127.0,
                                            op0=ALU.mult, op1=ALU.min)
                    qi = wk.tile([128, NLOC], I8, tag="qi")
                    nc.vector.tensor_scalar(qi[:], tq[:], -127.0, None, op0=ALU.max)
                    nc.sync.dma_start(out_d[s, o2 * 128:(o2 + 1) * 128, :], qi[:])
            nc.sync.dma_start(osc_d[:], osc[:])

    nc.compile()
    return nc


class _Runner:
    """Persistent shard_map jit over the 8 cores with device-cached weights."""

    def __init__(self, nc):
        import jax
        import jax.numpy as jnp
        import concourse.mybir as mybir
        from concourse import bass2jax
        from jax.experimental.shard_map import shard_map
        from jax.sharding import Mesh, NamedSharding, PartitionSpec

        bass2jax.install_neuronx_cc_hook()
        self.jax = jax
        self.nc = nc
        if nc.dbg_addr is not None and nc.dbg_callbacks:
            raise RuntimeError("debug callbacks unsupported in this dispatch path")

        partition_name = (nc.partition_id_tensor.name
                          if nc.partition_id_tensor is not None else None)
        dbg_name = nc.dbg_addr.name if nc.dbg_addr is not None else None

        in_params, in_specs_np = [], []
        out_names, out_avals, out_specs_np = [], [], []
        for alloc in nc.m.functions[0].allocations:
            if not isinstance(alloc, mybir.MemoryLocationSet):
                continue
            assert alloc.memorylocations
            name = alloc.memorylocations[0].name
            if alloc.kind == "ExternalInput":
                if name != partition_name:
                    in_params.append(name)
                    in_specs_np.append(
                        (tuple(alloc.tensor_shape), mybir.dt.np(alloc.dtype)))
            elif alloc.kind == "ExternalOutput":
                assert alloc.tensor_shape is not None and alloc.dtype is not None
                out_names.append(name)
                shape = tuple(alloc.tensor_shape)
                dtype = mybir.dt.np(alloc.dtype)
                out_avals.append(jax.core.ShapedArray(shape, dtype))
                out_specs_np.append((shape, dtype))
        self.in_params = in_params
        self.out_names = out_names
        self.out_specs_np = out_specs_np
        self.dbg_name = dbg_name
        n_params = len(in_params)
        n_outs = len(out_names)

        in_names_full = list(in_params) + list(out_names)
        if partition_name is not None:
            in_names_full.append(partition_name)

        devices = jax.devices()[:NCORE]
        assert len(devices) == NCORE
        self.mesh = Mesh(np.asarray(devices), ("core",))
        self.ns = NamedSharding(self.mesh, PartitionSpec("core"))

        def _body(*args):
            operands = list(args)
            if partition_name is not None:
                operands.append(bass2jax.partition_id_tensor())
            outs = bass2jax._bass_exec_p.bind(
                *operands,
                out_avals=tuple(out_avals),
                in_names=tuple(in_names_full),
                out_names=tuple(out_names),
                lowering_input_output_aliases=(),
                sim_require_finite=True,
                sim_require_nnan=True,
                nc=nc,
            )
            return tuple(outs)

        donate = tuple(range(n_params, n_params + n_outs))

        def _make_jit():
            return jax.jit(
                shard_map(
                    _body, mesh=self.mesh,
                    in_specs=(PartitionSpec("core"),) * (n_params + n_outs),
                    out_specs=(PartitionSpec("core"),) * n_outs,
                    check_rep=False,
                ),
                donate_argnums=donate,
                keep_unused=True,
            )

        self.sharded = _make_jit()
        # AOT-compile with bass_effect suppressed: the effectful dispatch
        # path costs an extra host<->device round trip per call. Falls back
        # to the plain jit at runtime if the fast path misbehaves.
        self.sharded_fast = None
        try:
            sds = [jax.ShapeDtypeStruct((NCORE * s[0], *s[1:]), d, sharding=self.ns)
                   for s, d in in_specs_np + out_specs_np]
            self.sharded_fast = bass2jax.fast_dispatch_compile(
                lambda: _make_jit().lower(*sds).compile())
        except Exception:
            self.sharded_fast = None

        zero_shardings = tuple(self.ns for _ in out_specs_np)

        def _mkzeros():
            return tuple(jnp.zeros((NCORE * s[0], *s[1:]), d)
                         for s, d in out_specs_np)

        self.zeros_fn = jax.jit(_mkzeros, out_shardings=zero_shardings)
        self.cached = {}
        # The NEFF writes every element of its outputs, so the donated
        # "zero" buffers never need to actually be zero: after the first
        # call, last call's (already fetched) output arrays are donated
        # back, avoiding the on-device zeros round trip.
        self._next_zs = None

    def set_weights(self, host_arrays):
        """host_arrays: name -> per-core array, replicated to all cores."""
        for name, arr in host_arrays.items():
            g = np.broadcast_to(arr, (NCORE,) + arr.shape)
            g = np.ascontiguousarray(g).reshape(NCORE * arr.shape[0], *arr.shape[1:])
            self.cached[name] = self.jax.device_put(g, self.ns)
        for v in self.cached.values():
            v.block_until_ready()

    def dispatch(self, per_call):
        """Asynchronously enqueue one NEFF execution; returns device arrays."""
        args = []
        for name in self.in_params:
            if name in self.cached:
                args.append(self.cached[name])
            elif name == self.dbg_name:
                args.append(np.zeros((NCORE, 2), np.uint32))
            else:
                args.append(per_call[name])
        zs = self._next_zs if self._next_zs is not None else self.zeros_fn()
        self._next_zs = None
        if self.sharded_fast is not None:
            try:
                return self.sharded_fast(*args, *zs)
            except Exception:
                # disable the fast path and redo with fresh donation buffers
                self.sharded_fast = None
                zs = self.zeros_fn()
        return self.sharded(*args, *zs)

    def finish(self, outs):
        """Fetch results to host; recycle the device buffers for donation."""
        res = self.jax.device_get(list(outs))
        self._next_zs = outs
        return res

    def discard(self, outs):
        """Drop an unwanted dispatch but keep its buffers for donation."""
        self._next_zs = outs

    def run(self, per_call):
        """per_call: name -> global (NCORE*dim0, ...) array. Returns out arrays."""
        import time
        last_err = None
        for attempt in range(4):
            try:
                return self.finish(self.dispatch(per_call))
            except Exception as e:  # transient NRT exec-unit faults: retry
                last_err = e
                self._next_zs = None
                time.sleep(0.5 * attempt)
        raise last_err


def _weights_fp(warrs):
    c = 0
    meta = []
    for a in warrs:
        a = np.ascontiguousarray(a)
        meta.append((a.shape, str(a.dtype)))
        c = zlib.crc32(a, c)
    return (c, tuple(meta))


def kernel(**inputs):
    desc0 = np.ascontiguousarray(inputs["desc0"], dtype=np.float32)
    desc1 = np.ascontiguousarray(inputs["desc1"], dtype=np.float32)
    warrs = [np.asarray(inputs[k]) for k in
             ["Wq", "bq", "Wk", "bk", "Wv", "bv", "Wm", "bm",
              "W1", "b1", "gamma", "beta", "W2", "b2"]]

    rn = _COMPILED.get("runner")
    if rn is None:
        rn = _Runner(_build())
        _COMPILED["runner"] = rn

    # core c = 4*b + h4 owns columns [h4*NLOC, (h4+1)*NLOC) of batch b
    m = np.empty((NCORE, 2, D, NLOC), np.float16)
    m[:, 0] = desc0.reshape(B, D, 4, NLOC).transpose(0, 2, 1, 3).reshape(NCORE, D, NLOC)
    m[:, 1] = desc1.reshape(B, D, 4, NLOC).transpose(0, 2, 1, 3).reshape(NCORE, D, NLOC)
    margs = {"m": m.reshape(NCORE * 2, D, NLOC)}

    # Optimistically dispatch with the device-cached weights and verify the
    # weight fingerprint while the NEFF runs; on mismatch (or first call)
    # upload the weights and re-run.
    outs = None
    if _COMPILED.get("wfp") is not None:
        try:
            pend = rn.dispatch(margs)
        except Exception:
            pend = None
        fp = _weights_fp(warrs)
        if pend is not None and fp == _COMPILED["wfp"]:
            try:
                outs = rn.finish(pend)
            except Exception:
                rn._next_zs = None
        elif pend is not None:
            rn.discard(pend)
    else:
        fp = _weights_fp(warrs)
    if outs is None:
        if _COMPILED.get("wfp") != fp:
            Wpack, BIAS = _prep_host(*warrs)
            rn.set_weights({"W": Wpack, "BIAS": BIAS})
            _COMPILED["wfp"] = fp
        outs = rn.run(margs)
    i_out = rn.out_names.index("out")
    i_osc = rn.out_names.index("osc")
    qv = outs[i_out].reshape(NCORE, 2, D, NLOC)
    rr = outs[i_osc].reshape(NCORE, 128, 2, 2)  # [core, row, stream, half]
    inv = np.ascontiguousarray(
        (1.0 / rr).transpose(0, 2, 3, 1)).reshape(NCORE, 2, D)

    o0 = np.empty((B, D, N), np.float32)
    o1 = np.empty((B, D, N), np.float32)
    for c in range(NCORE):
        b, h4 = c // 4, c % 4
        sl = slice(h4 * NLOC, (h4 + 1) * NLOC)
        np.multiply(qv[c, 0], inv[c, 0][:, None], out=o0[b][:, sl],
                    casting="unsafe")
        np.multiply(qv[c, 1], inv[c, 1][:, None], out=o1[b][:, sl],
                    casting="unsafe")
    return (o0, o1)
